# revision 27
# baseline (speedup 1.0000x reference)
"""Trainium2 Bass kernel for DBNN (double-exponential depthwise causal conv
+ zero-diag bilinear readout).

Math: the conv kernel k[n,d] = omega_n*(1-exp(-d/tau_rise_n))*exp(-d/tau_decay_n)
is a difference of two geometric series: k[n,d] = omega_n*(a_n^d - b_n^d) with
a_n = exp(-1/tau_decay_n), b_n = exp(-1/tau_rise_n - 1/tau_decay_n).
So y = x (*) k collapses to two first-order IIR filters:
    ya[t] = a*ya[t-1] + x[t]      (tensor_tensor_scan, mult/add)
    ybn[t] = b*ybn[t-1] - x[t]    (tensor_tensor_scan, mult/subtract -> -yb)
    u = ya + ybn = ya - yb,  y = omega * u   (omega folded into G and the
                                              linear term instead)
Readout per (batch,t):  out = u^T G u + omega^T u - 70
with G = diag(omega) @ (zero_diag(W)+zero_diag(W)^T)/2 @ diag(omega),
computed as Q = G^T u (PE matmuls), P = (Q + omega) .* u (scalar_tensor_tensor),
out = ones^T P (PE ones-matmul over partitions) - 70.

Sharding: data-parallel over batch B=32 across 8 cores (4 batches/core).
"""

import numpy as np

import concourse.mybir as mybir
from concourse import bacc, bass
from concourse.tile import TileContext
from concourse.bass_utils import run_bass_kernel_spmd

F32 = mybir.dt.float32
F32R = mybir.dt.float32r

B, N, T = 32, 256, 1024
NCORES = 8
NB = B // NCORES          # batches per core
NH = N // 128             # 128-partition halves of the channel dim
FB = 512                  # psum free-dim chunk (fp32 moving max / one bank)

# engine assignment knobs (tuned during optimization)
SCAN_A_ENGINE = "vector"
SCAN_B_ENGINE = "vector"
ADD_ENGINE = "vector"
USE_BCAST_AP = False       # stride-0 AP for scan decay operand


def _emit(nc: bass.Bass):
    x_d = nc.dram_tensor("x", [NB, N, T], F32, kind="ExternalInput").ap()
    g_d = nc.dram_tensor("g", [NH * NH, 128, 128], F32R, kind="ExternalInput").ap()
    c_d = nc.dram_tensor("consts", [N, 4], F32, kind="ExternalInput").ap()
    # sel[:, b*NB + m] = 1 if m == b else 0  (one-hot selector columns so the
    # per-batch partition reductions land on psum partition b)
    sel_d = nc.dram_tensor("sel", [128, NB * NB], F32R, kind="ExternalInput").ap()
    o_d = nc.dram_tensor("out", [NB, T], F32, kind="ExternalOutput").ap()

    mult = mybir.AluOpType.mult
    add = mybir.AluOpType.add
    sub = mybir.AluOpType.subtract
    Copy = mybir.ActivationFunctionType.Copy
    Ident = mybir.ActivationFunctionType.Identity

    def eng(name):
        return getattr(nc, name)

    with TileContext(nc) as tc:
        with (
            tc.tile_pool(name="cpool", bufs=1) as cpool,
            tc.tile_pool(name="xpool", bufs=3) as xpool,
            tc.tile_pool(name="ypool", bufs=3) as ypool,
            tc.tile_pool(name="upool", bufs=4) as upool,
            tc.tile_pool(name="ppool", bufs=4) as ppool,
            tc.tile_pool(name="qpool", bufs=2, space="PSUM") as qpool,
            tc.tile_pool(name="rpool", bufs=1, space="PSUM") as rpool,
            tc.tile_pool(name="opool", bufs=2) as opool,
        ):
            # --- constants ---
            gt = []
            for k in range(NH * NH):
                t_ = cpool.tile([128, 128], F32R, tag=f"g{k}")
                nc.sync.dma_start(out=t_, in_=g_d[k])
                gt.append(t_)
            sel = cpool.tile([128, NB * NB], F32R, tag="sel")
            nc.sync.dma_start(out=sel, in_=sel_d)
            ct = []
            for h in range(NH):
                t_ = cpool.tile([128, 4], F32, tag=f"c{h}")
                nc.sync.dma_start(out=t_, in_=c_d[h * 128:(h + 1) * 128, :])
                ct.append(t_)

            # Absorb the const-DMA completions into the DVE (and GPSIMD)
            # vector clocks with one tiny op each, so later scan/STT
            # instructions (wait-slot-limited ISA encodings) never need to
            # wait on the const DMA lanes directly.
            dummy = cpool.tile([128, 1], F32, tag="dummy")
            nc.vector.tensor_tensor(out=dummy[:, :], in0=ct[0][:, 3:4],
                                    in1=ct[1][:, 3:4], op=add)
            dummy2 = cpool.tile([128, 1], F32, tag="dummy2")
            nc.gpsimd.tensor_tensor(out=dummy2[:, :], in0=ct[0][:, 3:4],
                                    in1=ct[1][:, 3:4], op=add)

            if USE_BCAST_AP:
                a_bc = [ct[h][:, 0:1].broadcast_to([128, T]) for h in range(NH)]
                b_bc = [ct[h][:, 1:2].broadcast_to([128, T]) for h in range(NH)]
            else:
                a_bc, b_bc = [], []
                for h in range(NH):
                    ta = cpool.tile([128, T], F32, tag=f"abc{h}")
                    tb = cpool.tile([128, T], F32, tag=f"bbc{h}")
                    nc.gpsimd.memset(ta[:, :], 0.0)
                    nc.gpsimd.memset(tb[:, :], 0.0)
                    nc.scalar.activation(ta[:, :], ta[:, :], Ident,
                                         bias=ct[h][:, 0:1], scale=0.0)
                    nc.scalar.activation(tb[:, :], tb[:, :], Ident,
                                         bias=ct[h][:, 1:2], scale=0.0)
                    a_bc.append(ta[:, :])
                    b_bc.append(tb[:, :])

            # per-f accumulator tiles [NB, FB]: row b = output row for batch b
            rts = [rpool.tile([NB, FB], F32, tag=f"rt{f}", name=f"rt{f}")
                   for f in range(T // FB)]

            for b in range(NB):
                # --- conv: two IIR scans per channel-half ---
                uts = []
                for h in range(NH):
                    xt = xpool.tile([128, T], F32, tag="x")
                    nc.sync.dma_start(out=xt, in_=x_d[b, h * 128:(h + 1) * 128, :])
                    ya = ypool.tile([128, T], F32, tag="ya")
                    ybn = ypool.tile([128, T], F32, tag="ybn")
                    eng(SCAN_A_ENGINE).tensor_tensor_scan(
                        ya[:, :], a_bc[h], xt[:, :], 0.0, mult, add)
                    eng(SCAN_B_ENGINE).tensor_tensor_scan(
                        ybn[:, :], b_bc[h], xt[:, :], 0.0, mult, sub)
                    ut = upool.tile([128, T], F32R, tag="u")
                    eng(ADD_ENGINE).tensor_tensor(
                        out=ut[:, :], in0=ya[:, :], in1=ybn[:, :], op=add)
                    uts.append(ut)

                # --- readout ---
                for mh in range(NH):
                    qt = qpool.tile([128, T], F32, tag="q")
                    for f in range(T // FB):
                        fs = slice(f * FB, (f + 1) * FB)
                        for nh in range(NH):
                            nc.tensor.matmul(
                                qt[:, fs],
                                lhsT=gt[nh * NH + mh][:, :],
                                rhs=uts[nh][:, fs],
                                start=(nh == 0),
                                stop=(nh == NH - 1),
                            )
                    pt = ppool.tile([128, T], F32R, tag="p")
                    for f in range(T // FB):
                        fs = slice(f * FB, (f + 1) * FB)
                        nc.vector.scalar_tensor_tensor(
                            out=pt[:, fs], in0=qt[:, fs],
                            scalar=ct[mh][:, 2:3],
                            in1=uts[mh][:, fs].bitcast(F32),
                            op0=add, op1=mult)
                    if mh == 0:
                        pts = [pt]
                    else:
                        pts.append(pt)

                for f in range(T // FB):
                    fs = slice(f * FB, (f + 1) * FB)
                    for mh in range(NH):
                        nc.tensor.matmul(
                            rts[f][:, :],
                            lhsT=sel[:, b * NB:(b + 1) * NB],
                            rhs=pts[mh][:, fs],
                            start=(b == 0 and mh == 0),
                            stop=(b == NB - 1 and mh == NH - 1),
                            skip_group_check=True,
                        )

            # bias (-70) folded into the ScalarE copy, one DMA for all rows
            ot = opool.tile([NB, T], F32, tag="o")
            for f in range(T // FB):
                fs = slice(f * FB, (f + 1) * FB)
                nc.scalar.activation(ot[:, fs], rts[f][:, :], Copy, bias=-70.0)
            nc.sync.dma_start(out=o_d[:, :], in_=ot[:, :])


_CACHE = {}


def _build():
    if "nc" not in _CACHE:
        nc = bacc.Bacc("TRN2", target_bir_lowering=False, debug=False,
                       num_devices=NCORES)
        _emit(nc)
        nc.finalize()
        _CACHE["nc"] = nc
    return _CACHE["nc"]


def _host_prep(x, tau_rise, tau_decay, omega, W):
    x = np.ascontiguousarray(np.asarray(x, dtype=np.float32))
    tr = np.asarray(tau_rise, dtype=np.float64)
    td = np.asarray(tau_decay, dtype=np.float64)
    om = np.asarray(omega, dtype=np.float64)
    a = np.exp(-1.0 / td)
    b = np.exp(-1.0 / tr - 1.0 / td)
    Wm = np.asarray(W, dtype=np.float64)[0].copy()
    np.fill_diagonal(Wm, 0.0)
    G = (Wm + Wm.T) / 2.0
    G = om[:, None] * om[None, :] * G
    gblk = np.empty((NH * NH, 128, 128), dtype=np.float32)
    for nh in range(NH):
        for mh in range(NH):
            gblk[nh * NH + mh] = G[nh * 128:(nh + 1) * 128,
                                   mh * 128:(mh + 1) * 128]
    consts = np.empty((N, 4), dtype=np.float32)
    consts[:, 0] = a
    consts[:, 1] = b
    consts[:, 2] = om
    consts[:, 3] = 1.0
    return x, gblk, consts


def make_in_maps(x, tau_rise, tau_decay, omega, W):
    x, gblk, consts = _host_prep(x, tau_rise, tau_decay, omega, W)
    sel = np.zeros((128, NB * NB), dtype=np.float32)
    for b_ in range(NB):
        sel[:, b_ * NB + b_] = 1.0
    return [
        {"x": x[c * NB:(c + 1) * NB], "g": gblk, "consts": consts,
         "sel": sel}
        for c in range(NCORES)
    ]


def run(inputs, trace=False):
    nc = _build()
    in_maps = make_in_maps(**inputs)
    res = run_bass_kernel_spmd(nc, in_maps, list(range(NCORES)), trace=trace)
    out = np.concatenate([r["out"] for r in res.results], axis=0)
    return out.astype(np.float32), res


def kernel(x, tau_rise, tau_decay, omega, W):
    out, _ = run(dict(x=x, tau_rise=tau_rise, tau_decay=tau_decay,
                      omega=omega, W=W))
    return out
